# revision 7
# baseline (speedup 1.0000x reference)
"""Trainium2 Bass kernel for CIN layer:
    out[b,c,d] = sum_{h,m} W[c, h*M+m] * xk[b,h,d] * x0[b,m,d] + bias[c]

Shapes (hardcoded): x0 [512,40,64] f32, xk [512,128,64] f32,
W [128,5120] f32, b [128] f32 -> out [512,128,64] f32.

Strategy: data-parallel over batch B across 8 cores (64 batches/core).
Per core, columns are the 64*64=4096 (b,d) pairs. The 5120-long (h,m)
contraction is split into 40 chunks of 128 rows with a mixed-radix
partition layout: chunk (g, j) covers m in the 8-wide group g (5 groups)
x h in the 16-wide block j (8 blocks); partition p holds
(m = 8g + p//16, h = 16j + p%16). Then
  outer[p, col] = xkrep_j[p, col] * x0bc_g[p, col]  (DVE TT, bf16 2x)
  psum[q] += w3[t][p,c].T @ outer[:, q*512:...]     (PE, accum 40 chunks)
xkrep_j / x0bc_g replicas are produced host-side (pure layout).

The kernel is DVE-bound (the 5120x4096 elementwise outer products at
2 bf16/cycle/lane ~= 85us; PE matmuls are 68us) so everything is
organized to keep the DVE back-to-back: TTs are fused in j-pairs (40
instructions of [128,4096] instead of 80 of [128,2048], halving the
per-instruction SBUF-access overhead), input DMA issue alternates
between the Sync and Activation HWDGE queues so descriptor-gen
serialization (~0.9us per dma_start) never gates the DVE, and tile
halves are loaded in first-use order. Output is written c-major
([C,BC,D]) so each PSUM-bank store is one 2KB descriptor per
partition; the host transposes back. Bias-add is fused into the
PSUM->SBUF eviction (ACT for phase-0 banks mid-kernel; split ACT/DVE
at the tail where the DVE is free).
"""

import numpy as np
import ml_dtypes

B, M, H, D, C = 512, 40, 128, 64, 128
N_CORES = 8
BC = B // N_CORES          # 64 batches per core
COLS = BC * D              # 4096 (b,d) columns per core
NG = 8                     # PSUM banks
GW = COLS // NG            # 512 columns per bank
MG = 8                     # m-values per chunk group
NMG = M // MG              # 5 m-groups
HB = 128 // MG             # 16 h-values per block
NHB = H // HB              # 8 h-blocks
NCHUNK = NMG * NHB         # 40 contraction chunks
HC = COLS // 2             # 2048 columns per phase

# TT issue order: j-pairs outer, g inner. CK[t] = (g, j) of the t-th
# chunk consumed by the PE; w3 is laid out host-side in this order.
CK = [(g, 2 * jp + e) for jp in range(NHB // 2) for g in range(NMG)
      for e in range(2)]

_cache = {}


def _build(reps=1):
    import contextlib

    import concourse.bacc as bacc
    import concourse.mybir as mybir
    from concourse.tile import TileContext

    f32 = mybir.dt.float32
    bf16 = mybir.dt.bfloat16

    nc = bacc.Bacc("TRN2", debug=False, num_devices=N_CORES)

    xkr_d = nc.dram_tensor("xkrep_in", [NHB, 128, COLS], bf16, kind="ExternalInput")
    x0b_d = nc.dram_tensor("x0bc_in", [NMG, 128, COLS], bf16, kind="ExternalInput")
    w3_d = nc.dram_tensor("w3_in", [128, NCHUNK * C], bf16, kind="ExternalInput")
    bias_d = nc.dram_tensor("bias_in", [C, 1], f32, kind="ExternalInput")
    out_d = nc.dram_tensor("out", [C, BC, D], f32, kind="ExternalOutput")

    with TileContext(nc) as tc:
        with (
            tc.tile_pool(name="const", bufs=1) as cpool,
            tc.tile_pool(name="work", bufs=5) as wpool,
            tc.tile_pool(name="outp", bufs=4) as opool,
            tc.tile_pool(name="psum", bufs=1, space="PSUM") as ppool,
        ):
            # ---- persistent input tiles (single big allocations) ----
            xkr_sb = cpool.tile([128, NHB * COLS], bf16)
            x0b_sb = cpool.tile([128, NMG * COLS], bf16)
            w3_sb = cpool.tile([128, NCHUNK * C], bf16)
            bias_sb = cpool.tile([128, 1], f32)

            def xkr(j, ph):
                return xkr_sb[:, j * COLS + ph * HC:j * COLS + (ph + 1) * HC]

            def x0b(g, ph):
                return x0b_sb[:, g * COLS + ph * HC:g * COLS + (ph + 1) * HC]

            # ---- input DMA, two parallel HWDGE chains in first-use
            # order. The very first TT only needs the first quarter of
            # xkr block 0 and x0bc group 0, so those lead each chain as
            # quarter-tiles; w3 and bias are needed much later. Tile's
            # subtile tracking lets each TT wait only on what it reads.
            QC = HC // 2
            for q in range(2):
                nc.sync.dma_start(
                    out=xkr_sb[:, q * QC:(q + 1) * QC],
                    in_=xkr_d.ap()[0][:, q * QC:(q + 1) * QC])
                nc.scalar.dma_start(
                    out=x0b_sb[:, q * QC:(q + 1) * QC],
                    in_=x0b_d.ap()[0][:, q * QC:(q + 1) * QC])
            sync_plan = [("x", i, 0) for i in range(1, NHB)]
            sync_plan += [("b",)] + [("x", i, 1) for i in range(NHB)]
            scalar_plan = [("w", 0), ("0", 1, 0), ("0", 2, 0), ("0", 3, 0),
                           ("0", 4, 0), ("w", 1)]
            scalar_plan += [("0", i, 1) for i in range(NMG)]
            for eng, plan in ((nc.sync, sync_plan), (nc.scalar, scalar_plan)):
                for item in plan:
                    if item[0] == "b":
                        eng.dma_start(out=bias_sb, in_=bias_d.ap())
                    elif item[0] == "w":
                        hw = NCHUNK * C // 2
                        eng.dma_start(
                            out=w3_sb[:, item[1] * hw:(item[1] + 1) * hw],
                            in_=w3_d.ap()[:, item[1] * hw:(item[1] + 1) * hw])
                    elif item[0] == "x":
                        _, i, ph = item
                        eng.dma_start(out=xkr(i, ph),
                                      in_=xkr_d.ap()[i][:, ph * HC:(ph + 1) * HC])
                    else:
                        _, i, ph = item
                        eng.dma_start(out=x0b(i, ph),
                                      in_=x0b_d.ap()[i][:, ph * HC:(ph + 1) * HC])

            loop_ctx = (
                tc.For_i(
                    0, reps, 1,
                    hint_engines=(mybir.EngineType.PE,),
                    staggered_reset=True,
                )
                if reps > 1
                else contextlib.nullcontext()
            )
            with loop_ctx:
                psums = []
                for q in range(NG):
                    ps = ppool.tile([128, GW], f32, name=f"ps{q}", tag=f"ps{q}")
                    psums.append(ps)

                if reps == 1:
                    # Warm the PE's HAM clock-gate with small dummy
                    # matmuls on scratch data while the prologue DMAs
                    # are in flight. Real first-accumulation MMs use
                    # start=True, so PSUM garbage is discarded.
                    scratch = cpool.tile([128, 128], bf16)
                    nc.gpsimd.memset(scratch, 0.0)
                    for _ in range(28):
                        nc.tensor.matmul(
                            psums[0][:, :128],
                            lhsT=scratch,
                            rhs=scratch,
                            start=True,
                            stop=True,
                        )

                # ---- main loop: 2 column phases over 20 fused TTs ----
                # Fused TT t covers chunks CK[2t], CK[2t+1] (same g,
                # adjacent j): in0 reads the two xkrep j-blocks via a
                # strided outer dim, in1 reads x0bc_g twice (stride-0
                # outer dim), keeping the DVE in 2x bf16 mode. The PE
                # consumes each fused outer as 2 chunks x 4 banks of
                # 512 columns, accumulating 40 chunks per bank.
                out_ap = out_d.ap()
                bpg = BC // NG  # batches per bank
                for ph in range(2):
                    for t in range(0, NCHUNK, 2):
                        g, j0 = CK[t]
                        outer = wpool.tile(
                            [128, 2 * HC], bf16, name=f"outer{ph}_{t}",
                            tag=f"outer{(t // 2) % 5}", bufs=1,
                        )
                        if ph == 0 and t == 0:
                            # Warm-up path: single-chunk TTs (the first
                            # in quarter-tiles) so the DVE starts as
                            # soon as the first quarter DMAs land.
                            QC = HC // 2
                            for c0, c1 in ((0, QC), (QC, HC)):
                                nc.vector.tensor_mul(
                                    outer[:, c0:c1],
                                    xkr_sb[:, c0:c1],
                                    x0b_sb[:, c0:c1])
                            nc.vector.tensor_mul(
                                outer[:, HC:2 * HC],
                                xkr_sb[:, COLS:COLS + HC],
                                x0b_sb[:, :HC])
                        else:
                            # two j-blocks: strided outer dim on in0;
                            # stride-0 outer dim on in1 (same g twice)
                            in0 = (xkr_sb[:, j0 * COLS:(j0 + 2) * COLS]
                                   .rearrange("p (two c) -> p two c", two=2)
                                   [:, :, ph * HC:(ph + 1) * HC])
                            in1 = (x0b(g, ph).unsqueeze(1)
                                   .broadcast_to([128, 2, HC]))
                            nc.vector.tensor_tensor(
                                outer.rearrange("p (two c) -> p two c", two=2),
                                in0,
                                in1,
                                mybir.AluOpType.mult,
                            )
                        for ql in range(NG // 2):
                            qb = ph * (NG // 2) + ql
                            for e in range(2):
                                k = t + e
                                nc.tensor.matmul(
                                    psums[qb],
                                    lhsT=w3_sb[:, k * C:(k + 1) * C],
                                    rhs=outer[:, e * HC + ql * GW:
                                              e * HC + (ql + 1) * GW],
                                    start=(k == 0),
                                    stop=(k == NCHUNK - 1),
                                )
                    # bias add + store for this phase's banks.
                    # Phase 0: ACT only (DVE is mid-stream). Phase 1:
                    # alternate ACT / DVE - the DVE is idle after its
                    # last TT - with the store DMAs alternating between
                    # the scalar and sync HWDGE queues so the two
                    # evict+store chains run fully in parallel.
                    for ql in range(NG // 2):
                        qb = ph * (NG // 2) + ql
                        out_sb = opool.tile(
                            [128, GW], f32, name=f"osb{qb}", tag="osb"
                        )
                        if ph == 1 and ql % 2 == 1:
                            nc.vector.tensor_scalar_add(
                                out_sb, psums[qb], bias_sb[:, 0:1])
                        else:
                            nc.scalar.activation(
                                out_sb,
                                psums[qb],
                                mybir.ActivationFunctionType.Identity,
                                bias=bias_sb[:, 0:1],
                                scale=1.0,
                            )
                        dma_eng = nc.scalar if ql % 2 == 0 else nc.sync
                        dma_eng.dma_start(
                            out=out_ap[:, qb * bpg:(qb + 1) * bpg, :],
                            in_=out_sb)

    nc.compile()
    return nc


def _prep_host(x0, xk, W, b):
    """Host-side layout prep (no arithmetic): shard, transpose, replicate."""
    part = np.arange(128)
    hh = (part % HB)[None, :] + HB * np.arange(NHB)[:, None]   # [NHB, 128]
    mm = (part // HB)[None, :] + MG * np.arange(NMG)[:, None]  # [NMG, 128]

    Wr = W.reshape(C, H, M)
    # w3[t] = weights for chunk CK[t], laid out [128, NCHUNK*C] so the
    # DMA is contiguous per partition and lhsT slices follow TT order.
    w3 = np.empty((128, NCHUNK * C), ml_dtypes.bfloat16)
    for t, (g, j) in enumerate(CK):
        w3[:, t * C:(t + 1) * C] = Wr[:, hh[j], mm[g]].T.astype(
            ml_dtypes.bfloat16)
    bias = np.ascontiguousarray(b.reshape(C, 1)).astype(np.float32)

    in_maps = []
    for k in range(N_CORES):
        x0s = x0[k * BC:(k + 1) * BC]            # [BC, M, D]
        xks = xk[k * BC:(k + 1) * BC]            # [BC, H, D]
        xk2 = (
            np.ascontiguousarray(xks.transpose(1, 0, 2))
            .reshape(H, COLS)
            .astype(ml_dtypes.bfloat16)
        )
        x02 = (
            np.ascontiguousarray(x0s.transpose(1, 0, 2))
            .reshape(M, COLS)
            .astype(ml_dtypes.bfloat16)
        )
        in_maps.append(
            {
                "xkrep_in": np.ascontiguousarray(xk2[hh]),
                "x0bc_in": np.ascontiguousarray(x02[mm]),
                "w3_in": w3,
                "bias_in": bias,
            }
        )
    return in_maps


def _run(in_maps, **kwargs):
    from concourse import bass_utils

    if "nc" not in _cache:
        _cache["nc"] = _build()
    return bass_utils.run_bass_kernel_spmd(
        _cache["nc"], in_maps, core_ids=list(range(N_CORES)), **kwargs
    )


def kernel(x0, xk, W, b, _bench=[None]):
    x0 = np.asarray(x0, dtype=np.float32)
    xk = np.asarray(xk, dtype=np.float32)
    W = np.asarray(W, dtype=np.float32)
    b = np.asarray(b, dtype=np.float32)
    in_maps = _prep_host(x0, xk, W, b)
    res = _run(in_maps)
    _bench[0] = res
    # per-core out is [C, BC, D]; concatenate batches then put C second
    out = np.concatenate([r["out"] for r in res.results], axis=1)
    return np.ascontiguousarray(out.transpose(1, 0, 2)).astype(
        np.float32, copy=False)


# revision 15
# speedup vs baseline: 1.1967x; 1.1967x over previous
"""Trainium2 Bass kernel for CIN layer:
    out[b,c,d] = sum_{h,m} W[c, h*M+m] * xk[b,h,d] * x0[b,m,d] + bias[c]

Shapes (hardcoded): x0 [512,40,64] f32, xk [512,128,64] f32,
W [128,5120] f32, b [128] f32 -> out [512,128,64] f32.

Strategy: data-parallel over batch B across 8 cores (64 batches/core).
Per core, columns are the 64*64=4096 (b,d) pairs. The 5120-long (h,m)
contraction is split into 40 chunks of 128 rows with a mixed-radix
partition layout: chunk (g, j) covers m in the 8-wide group g (5
groups) x h in the 16-wide block j (8 blocks); partition p holds
(m = 8g + p//16, h = 16j + p%16). Then
  outer[p, col] = xkrep_j[p, col] * x0bc_g[p, col]  (DVE TT, bf16 2x)
  psum[q] += w3[k][p,c].T @ outer[:, q*512:...]     (PE, accum 40 chunks)
xkrep_j / x0bc_g replicas are produced host-side (pure layout).

The kernel is DVE-bound (the 5120x4096 elementwise outer products at
2 bf16/cycle/lane ~= 85us; PE matmuls are 68us), so everything is
organized around an uninterrupted DVE stream:
 - TTs are batched per the GROUPS schedule: single-chunk TTs at the
   stream edges (earlier start on fewer input tiles; short PE+evict
   tail after the last TT) and j-pairs in steady state. A strided
   outer AP dim reads `size` adjacent xkrep j-blocks, a stride-0 dim
   reuses one x0bc block, keeping the DVE in 2x bf16 mode. Measured:
   back-to-back TTs fully amortize per-instruction overhead, so wider
   batches gain nothing.
 - The chunk order CK visits j-pairs outer / g inner so the first TTs
   need only xkr blocks 0-1 plus the x0bc groups, matching DMA
   delivery order (the prologue is DMA-delivery-bound at ~360GB/s
   across two queues).
 - Input DMA issue alternates between the Sync and Activation HWDGE
   queues in first-use order with big per-half transfers (fragmenting
   loads measurably slows total DMA); w3 is loaded in quarters in PE
   consumption order; its host layout is [128, 40*C] in CK order so
   each load is one contiguous descriptor per partition.
 - Bias-add is fused into the PSUM->SBUF eviction: ACT for phase-0
   banks mid-kernel, split ACT/DVE at the tail where the DVE is free;
   store DMAs alternate scalar/sync. Output is written c-major
   ([C,BC,D], one 2KB descriptor per partition per bank); the host
   transposes back.
"""

import numpy as np
import ml_dtypes

B, M, H, D, C = 512, 40, 128, 64, 128
N_CORES = 8
BC = B // N_CORES          # 64 batches per core
COLS = BC * D              # 4096 (b,d) columns per core
NG = 8                     # PSUM banks
GW = COLS // NG            # 512 columns per bank
MG = 8                     # m-values per chunk group
NMG = M // MG              # 5 m-groups
HB = 128 // MG             # 16 h-values per block
NHB = H // HB              # 8 h-blocks
NCHUNK = NMG * NHB         # 40 contraction chunks
HC = COLS // 2             # 2048 columns per phase

# Chunk consumption order: j-pairs outer, g inner - so the first TTs
# only need xkr blocks 0-1 and the x0bc groups, in load order.
CK = [(g, 2 * jp + e) for jp in range(NHB // 2) for g in range(NMG)
      for e in range(2)]
# TT batching per phase: sizes partition CK into same-g consecutive-j
# runs. Singles at the start (the very first in quarter-columns, so
# the DVE starts on the first quarter DMAs) and at the end (short
# PE+evict tail); pairs in steady state (back-to-back TTs amortize
# per-instruction overhead fully, so wider batches gain nothing).
GROUPS = [1, 1] + [2] * 18 + [1, 1]
assert sum(GROUPS) == NCHUNK

_cache = {}


def _build(reps=1):
    import contextlib

    import concourse.bacc as bacc
    import concourse.mybir as mybir
    from concourse.tile import TileContext

    f32 = mybir.dt.float32
    bf16 = mybir.dt.bfloat16

    nc = bacc.Bacc("TRN2", debug=False, num_devices=N_CORES)

    xkr_d = nc.dram_tensor("xkrep_in", [NHB, 128, COLS], bf16, kind="ExternalInput")
    x0b_d = nc.dram_tensor("x0bc_in", [NMG, 128, COLS], bf16, kind="ExternalInput")
    w3_d = nc.dram_tensor("w3_in", [128, NCHUNK * C], bf16, kind="ExternalInput")
    bias_d = nc.dram_tensor("bias_in", [C, 1], f32, kind="ExternalInput")
    out_d = nc.dram_tensor("out", [C, BC, D], f32, kind="ExternalOutput")

    with TileContext(nc) as tc:
        with (
            tc.tile_pool(name="const", bufs=1) as cpool,
            tc.tile_pool(name="work", bufs=4) as wpool,
            tc.tile_pool(name="outp", bufs=4) as opool,
            tc.tile_pool(name="psum", bufs=1, space="PSUM") as ppool,
        ):
            # ---- persistent input tiles (single big allocations) ----
            xkr_sb = cpool.tile([128, NHB * COLS], bf16)
            x0b_sb = cpool.tile([128, NMG * COLS], bf16)
            w3_sb = cpool.tile([128, NCHUNK * C], bf16)
            bias_sb = cpool.tile([128, 1], f32)

            def xkr(j, ph):
                return xkr_sb[:, j * COLS + ph * HC:j * COLS + (ph + 1) * HC]

            def x0b(g, ph):
                return x0b_sb[:, g * COLS + ph * HC:g * COLS + (ph + 1) * HC]

            # ---- input DMA: two parallel HWDGE chains, first-use
            # order. Chunk 0's inputs are quartered and crossed over
            # the queues so the first TTs can start after ~0.5MB of
            # traffic; w3 quarters follow PE consumption; phase-1
            # halves and bias trail (needed much later).
            WQ = NCHUNK * C // 4

            def dx(eng, i, ph):
                eng.dma_start(out=xkr(i, ph),
                              in_=xkr_d.ap()[i][:, ph * HC:(ph + 1) * HC])

            def d0(eng, i, ph):
                eng.dma_start(out=x0b(i, ph),
                              in_=x0b_d.ap()[i][:, ph * HC:(ph + 1) * HC])

            def dw(eng, q):
                eng.dma_start(out=w3_sb[:, q * WQ:(q + 1) * WQ],
                              in_=w3_d.ap()[:, q * WQ:(q + 1) * WQ])

            sy, sc = nc.sync, nc.scalar
            dx(sy, 0, 0)
            d0(sc, 0, 0)
            dx(sy, 1, 0)
            dw(sc, 0)
            d0(sy, 3, 0)
            d0(sc, 1, 0)
            dx(sy, 2, 0)
            d0(sc, 2, 0)
            dx(sy, 3, 0)
            d0(sc, 4, 0)
            dx(sy, 5, 0)
            dw(sc, 1)
            dx(sy, 7, 0)
            dx(sc, 4, 0)
            nc.sync.dma_start(out=bias_sb, in_=bias_d.ap())
            dx(sc, 6, 0)
            dw(sy, 2)
            dw(sc, 3)
            # phase-1 halves, first-use order, alternating
            ph1_order = [("x", 0), ("0", 0), ("x", 1), ("0", 1), ("0", 2),
                         ("x", 2), ("x", 3), ("0", 3), ("0", 4), ("x", 4),
                         ("x", 5), ("x", 6), ("x", 7)]
            for n, (kind, i) in enumerate(ph1_order):
                eng = (sy, sc)[n % 2]
                (dx if kind == "x" else d0)(eng, i, 1)

            loop_ctx = (
                tc.For_i(
                    0, reps, 1,
                    hint_engines=(mybir.EngineType.PE,),
                    staggered_reset=True,
                )
                if reps > 1
                else contextlib.nullcontext()
            )
            with loop_ctx:
                psums = []
                for q in range(NG):
                    ps = ppool.tile([128, GW], f32, name=f"ps{q}", tag=f"ps{q}")
                    psums.append(ps)

                if reps == 1:
                    # Warm the PE's HAM clock-gate with small dummy
                    # matmuls on scratch data while the prologue DMAs
                    # are in flight. Real first-accumulation MMs use
                    # start=True, so PSUM garbage is discarded.
                    scratch = cpool.tile([128, 128], bf16)
                    nc.gpsimd.memset(scratch, 0.0)
                    for _ in range(28):
                        nc.tensor.matmul(
                            psums[0][:, :128],
                            lhsT=scratch,
                            rhs=scratch,
                            start=True,
                            stop=True,
                        )

                # ---- main loop: 2 column phases over the TT groups ----
                out_ap = out_d.ap()
                bpg = BC // NG  # batches per bank
                for ph in range(2):
                    t0 = 0
                    for gi, size in enumerate(GROUPS):
                        g, j0 = CK[t0]
                        outer = wpool.tile(
                            [128, size * HC], bf16, name=f"outer{ph}_{t0}",
                            tag=f"outer{gi % 4}", bufs=1,
                        )
                        if size == 1:
                            nc.vector.tensor_mul(
                                outer, xkr(j0, ph), x0b(g, ph))
                        else:
                            # `size` adjacent j-blocks: strided outer
                            # dim on in0; stride-0 outer dim on in1
                            in0 = (xkr_sb[:, j0 * COLS:(j0 + size) * COLS]
                                   .rearrange("p (s c) -> p s c", s=size)
                                   [:, :, ph * HC:(ph + 1) * HC])
                            in1 = (x0b(g, ph).unsqueeze(1)
                                   .broadcast_to([128, size, HC]))
                            nc.vector.tensor_tensor(
                                outer.rearrange("p (s c) -> p s c", s=size),
                                in0,
                                in1,
                                mybir.AluOpType.mult,
                            )
                        for ql in range(NG // 2):
                            qb = ph * (NG // 2) + ql
                            for e in range(size):
                                k = t0 + e
                                nc.tensor.matmul(
                                    psums[qb],
                                    lhsT=w3_sb[:, k * C:(k + 1) * C],
                                    rhs=outer[:, e * HC + ql * GW:
                                              e * HC + (ql + 1) * GW],
                                    start=(k == 0),
                                    stop=(k == NCHUNK - 1),
                                )
                        t0 += size
                    # bias add + store for this phase's banks.
                    # Phase 0: ACT only (DVE is mid-stream). Phase 1:
                    # alternate ACT / DVE (free after its last TT);
                    # store DMAs alternate scalar/sync so the two
                    # evict+store chains run in parallel.
                    for ql in range(NG // 2):
                        qb = ph * (NG // 2) + ql
                        out_sb = opool.tile(
                            [128, GW], f32, name=f"osb{qb}", tag="osb"
                        )
                        if ph == 1 and ql % 2 == 1:
                            nc.vector.tensor_scalar_add(
                                out_sb, psums[qb], bias_sb[:, 0:1])
                        else:
                            nc.scalar.activation(
                                out_sb,
                                psums[qb],
                                mybir.ActivationFunctionType.Identity,
                                bias=bias_sb[:, 0:1],
                                scale=1.0,
                            )
                        dma_eng = nc.scalar if ql % 2 == 0 else nc.sync
                        dma_eng.dma_start(
                            out=out_ap[:, qb * bpg:(qb + 1) * bpg, :],
                            in_=out_sb)

    nc.compile()
    return nc


def _prep_host(x0, xk, W, b):
    """Host-side layout prep (no arithmetic): shard, transpose, replicate."""
    part = np.arange(128)
    hh = (part % HB)[None, :] + HB * np.arange(NHB)[:, None]   # [NHB, 128]
    mm = (part // HB)[None, :] + MG * np.arange(NMG)[:, None]  # [NMG, 128]

    Wr = W.reshape(C, H, M)
    # w3[:, k*C:(k+1)*C] = lhsT for chunk CK[k]; [128, NCHUNK*C] layout
    # so each load quarter is contiguous per partition.
    w3 = np.empty((128, NCHUNK * C), ml_dtypes.bfloat16)
    for k, (g, j) in enumerate(CK):
        w3[:, k * C:(k + 1) * C] = Wr[:, hh[j], mm[g]].T.astype(
            ml_dtypes.bfloat16)
    bias = np.ascontiguousarray(b.reshape(C, 1)).astype(np.float32)

    in_maps = []
    for kcore in range(N_CORES):
        x0s = x0[kcore * BC:(kcore + 1) * BC]    # [BC, M, D]
        xks = xk[kcore * BC:(kcore + 1) * BC]    # [BC, H, D]
        xk2 = (
            np.ascontiguousarray(xks.transpose(1, 0, 2))
            .reshape(H, COLS)
            .astype(ml_dtypes.bfloat16)
        )
        x02 = (
            np.ascontiguousarray(x0s.transpose(1, 0, 2))
            .reshape(M, COLS)
            .astype(ml_dtypes.bfloat16)
        )
        in_maps.append(
            {
                "xkrep_in": np.ascontiguousarray(xk2[hh]),
                "x0bc_in": np.ascontiguousarray(x02[mm]),
                "w3_in": w3,
                "bias_in": bias,
            }
        )
    return in_maps


def _run(in_maps, **kwargs):
    from concourse import bass_utils

    if "nc" not in _cache:
        _cache["nc"] = _build()
    return bass_utils.run_bass_kernel_spmd(
        _cache["nc"], in_maps, core_ids=list(range(N_CORES)), **kwargs
    )


def kernel(x0, xk, W, b, _bench=[None]):
    x0 = np.asarray(x0, dtype=np.float32)
    xk = np.asarray(xk, dtype=np.float32)
    W = np.asarray(W, dtype=np.float32)
    b = np.asarray(b, dtype=np.float32)
    in_maps = _prep_host(x0, xk, W, b)
    res = _run(in_maps)
    _bench[0] = res
    # per-core out is [C, BC, D]; concatenate batches then put C second
    out = np.concatenate([r["out"] for r in res.results], axis=1)
    return np.ascontiguousarray(out.transpose(1, 0, 2)).astype(
        np.float32, copy=False)


# revision 17
# speedup vs baseline: 1.2017x; 1.0042x over previous
"""Trainium2 Bass kernel for CIN layer:
    out[b,c,d] = sum_{h,m} W[c, h*M+m] * xk[b,h,d] * x0[b,m,d] + bias[c]

Shapes (hardcoded): x0 [512,40,64] f32, xk [512,128,64] f32,
W [128,5120] f32, b [128] f32 -> out [512,128,64] f32.

Strategy: data-parallel over batch B across 8 cores (64 batches/core).
Per core, columns are the 64*64=4096 (b,d) pairs. The 5120-long (h,m)
contraction is split into 40 chunks of 128 rows with a mixed-radix
partition layout: chunk (g, j) covers m in the 8-wide group g (5
groups) x h in the 16-wide block j (8 blocks); partition p holds
(m = 8g + p//16, h = 16j + p%16). Then
  outer[p, col] = xkrep_j[p, col] * x0bc_g[p, col]  (DVE TT, bf16 2x)
  psum[q] += w3[k][p,c].T @ outer[:, q*512:...]     (PE, accum 40 chunks)
xkrep_j / x0bc_g replicas are produced host-side (pure layout).

The kernel is DVE-bound (the 5120x4096 elementwise outer products at
2 bf16/cycle/lane ~= 85us; PE matmuls are 68us), so everything is
organized around an uninterrupted DVE stream:
 - TTs are batched per the GROUPS schedule: single-chunk TTs at the
   stream edges (earlier start on fewer input tiles; short PE+evict
   tail after the last TT) and j-pairs in steady state. A strided
   outer AP dim reads `size` adjacent xkrep j-blocks, a stride-0 dim
   reuses one x0bc block, keeping the DVE in 2x bf16 mode. Measured:
   back-to-back TTs fully amortize per-instruction overhead, so wider
   batches gain nothing.
 - The chunk order CK visits j-pairs outer / g inner so the first TTs
   need only xkr blocks 0-1 plus the x0bc groups, matching DMA
   delivery order (the prologue is DMA-delivery-bound at ~360GB/s
   across two queues).
 - Input DMA issue alternates between the Sync and Activation HWDGE
   queues in first-use order with big per-half transfers (fragmenting
   loads measurably slows total DMA); w3 is loaded in quarters in PE
   consumption order; its host layout is [128, 40*C] in CK order so
   each load is one contiguous descriptor per partition.
 - Bias-add is fused into the PSUM->SBUF eviction: ACT for phase-0
   banks mid-kernel, split ACT/DVE at the tail where the DVE is free;
   store DMAs alternate scalar/sync. Output is written c-major
   ([C,BC,D], one 2KB descriptor per partition per bank); the host
   transposes back.
"""

import numpy as np
import ml_dtypes

B, M, H, D, C = 512, 40, 128, 64, 128
N_CORES = 8
BC = B // N_CORES          # 64 batches per core
COLS = BC * D              # 4096 (b,d) columns per core
NG = 8                     # PSUM banks
GW = COLS // NG            # 512 columns per bank
MG = 8                     # m-values per chunk group
NMG = M // MG              # 5 m-groups
HB = 128 // MG             # 16 h-values per block
NHB = H // HB              # 8 h-blocks
NCHUNK = NMG * NHB         # 40 contraction chunks
HC = COLS // 2             # 2048 columns per phase

# Chunk consumption order: j-pairs outer, g inner - so the first TTs
# only need xkr blocks 0-1 and the x0bc groups, in load order.
CK = [(g, 2 * jp + e) for jp in range(NHB // 2) for g in range(NMG)
      for e in range(2)]
# TT batching per phase: sizes partition CK into same-g consecutive-j
# runs. Singles at the start (the very first in quarter-columns, so
# the DVE starts on the first quarter DMAs) and at the end (short
# PE+evict tail); pairs in steady state (back-to-back TTs amortize
# per-instruction overhead fully, so wider batches gain nothing).
GROUPS = [1, 1] + [2] * 18 + [1, 1]
assert sum(GROUPS) == NCHUNK

_cache = {}


def _build(reps=1):
    import contextlib

    import concourse.bacc as bacc
    import concourse.mybir as mybir
    from concourse.tile import TileContext

    f32 = mybir.dt.float32
    bf16 = mybir.dt.bfloat16

    nc = bacc.Bacc("TRN2", debug=False, num_devices=N_CORES)

    xkr_d = nc.dram_tensor("xkrep_in", [NHB, 128, COLS], bf16, kind="ExternalInput")
    x0b_d = nc.dram_tensor("x0bc_in", [NMG, 128, COLS], bf16, kind="ExternalInput")
    w3_d = nc.dram_tensor("w3_in", [128, NCHUNK * C], bf16, kind="ExternalInput")
    bias_d = nc.dram_tensor("bias_in", [C, 1], f32, kind="ExternalInput")
    out_d = nc.dram_tensor("out", [C, BC, D], f32, kind="ExternalOutput")

    with TileContext(nc) as tc:
        with (
            tc.tile_pool(name="const", bufs=1) as cpool,
            tc.tile_pool(name="work", bufs=4) as wpool,
            tc.tile_pool(name="outp", bufs=4) as opool,
            tc.tile_pool(name="psum", bufs=1, space="PSUM") as ppool,
        ):
            # ---- persistent input tiles (single big allocations) ----
            xkr_sb = cpool.tile([128, NHB * COLS], bf16)
            x0b_sb = cpool.tile([128, NMG * COLS], bf16)
            w3_sb = cpool.tile([128, NCHUNK * C], bf16)
            bias_sb = cpool.tile([128, 1], f32)

            def xkr(j, ph):
                return xkr_sb[:, j * COLS + ph * HC:j * COLS + (ph + 1) * HC]

            def x0b(g, ph):
                return x0b_sb[:, g * COLS + ph * HC:g * COLS + (ph + 1) * HC]

            # ---- input DMA: two parallel HWDGE chains, first-use
            # order. Chunk 0's inputs are quartered and crossed over
            # the queues so the first TTs can start after ~0.5MB of
            # traffic; w3 quarters follow PE consumption; phase-1
            # halves and bias trail (needed much later).
            WQ = NCHUNK * C // 4

            def dx(eng, i, ph):
                eng.dma_start(out=xkr(i, ph),
                              in_=xkr_d.ap()[i][:, ph * HC:(ph + 1) * HC])

            def d0(eng, i, ph):
                eng.dma_start(out=x0b(i, ph),
                              in_=x0b_d.ap()[i][:, ph * HC:(ph + 1) * HC])

            def dw(eng, q):
                eng.dma_start(out=w3_sb[:, q * WQ:(q + 1) * WQ],
                              in_=w3_d.ap()[:, q * WQ:(q + 1) * WQ])

            # Queue plans interleave tiles across the two chains in
            # exact TT consumption order (xkr0+x0b0, xkr1, x0b1, x0b2,
            # x0b3, x0b4, xkr2+xkr3, ..), so each tile lands just
            # before its TT; w3 quarters slot into the slack.
            sy, sc = nc.sync, nc.scalar
            dx(sy, 0, 0)
            d0(sc, 0, 0)
            dx(sy, 1, 0)
            d0(sc, 1, 0)
            d0(sy, 2, 0)
            d0(sc, 3, 0)
            d0(sy, 4, 0)
            dx(sc, 2, 0)
            dx(sy, 3, 0)
            dw(sc, 0)
            dx(sy, 5, 0)
            dx(sc, 4, 0)
            dx(sy, 7, 0)
            dw(sc, 1)
            nc.sync.dma_start(out=bias_sb, in_=bias_d.ap())
            dx(sc, 6, 0)
            dw(sy, 2)
            dw(sc, 3)
            # phase-1 halves, first-use order, alternating
            ph1_order = [("x", 0), ("0", 0), ("x", 1), ("0", 1), ("0", 2),
                         ("x", 2), ("x", 3), ("0", 3), ("0", 4), ("x", 4),
                         ("x", 5), ("x", 6), ("x", 7)]
            for n, (kind, i) in enumerate(ph1_order):
                eng = (sy, sc)[n % 2]
                (dx if kind == "x" else d0)(eng, i, 1)

            loop_ctx = (
                tc.For_i(
                    0, reps, 1,
                    hint_engines=(mybir.EngineType.PE,),
                    staggered_reset=True,
                )
                if reps > 1
                else contextlib.nullcontext()
            )
            with loop_ctx:
                psums = []
                for q in range(NG):
                    ps = ppool.tile([128, GW], f32, name=f"ps{q}", tag=f"ps{q}")
                    psums.append(ps)

                if reps == 1:
                    # Warm the PE's HAM clock-gate with small dummy
                    # matmuls on scratch data while the prologue DMAs
                    # are in flight. Real first-accumulation MMs use
                    # start=True, so PSUM garbage is discarded.
                    scratch = cpool.tile([128, 128], bf16)
                    nc.gpsimd.memset(scratch, 0.0)
                    for _ in range(28):
                        nc.tensor.matmul(
                            psums[0][:, :128],
                            lhsT=scratch,
                            rhs=scratch,
                            start=True,
                            stop=True,
                        )

                # ---- main loop: 2 column phases over the TT groups ----
                out_ap = out_d.ap()
                bpg = BC // NG  # batches per bank
                for ph in range(2):
                    t0 = 0
                    for gi, size in enumerate(GROUPS):
                        g, j0 = CK[t0]
                        outer = wpool.tile(
                            [128, size * HC], bf16, name=f"outer{ph}_{t0}",
                            tag=f"outer{gi % 8}", bufs=1,
                        )
                        if size == 1:
                            nc.vector.tensor_mul(
                                outer, xkr(j0, ph), x0b(g, ph))
                        else:
                            # `size` adjacent j-blocks: strided outer
                            # dim on in0; stride-0 outer dim on in1
                            in0 = (xkr_sb[:, j0 * COLS:(j0 + size) * COLS]
                                   .rearrange("p (s c) -> p s c", s=size)
                                   [:, :, ph * HC:(ph + 1) * HC])
                            in1 = (x0b(g, ph).unsqueeze(1)
                                   .broadcast_to([128, size, HC]))
                            nc.vector.tensor_tensor(
                                outer.rearrange("p (s c) -> p s c", s=size),
                                in0,
                                in1,
                                mybir.AluOpType.mult,
                            )
                        for ql in range(NG // 2):
                            qb = ph * (NG // 2) + ql
                            for e in range(size):
                                k = t0 + e
                                nc.tensor.matmul(
                                    psums[qb],
                                    lhsT=w3_sb[:, k * C:(k + 1) * C],
                                    rhs=outer[:, e * HC + ql * GW:
                                              e * HC + (ql + 1) * GW],
                                    start=(k == 0),
                                    stop=(k == NCHUNK - 1),
                                )
                        t0 += size
                    # bias add + store for this phase's banks.
                    # Phase 0: ACT only (DVE is mid-stream). Phase 1:
                    # alternate ACT / DVE (free after its last TT);
                    # store DMAs alternate scalar/sync so the two
                    # evict+store chains run in parallel.
                    for ql in range(NG // 2):
                        qb = ph * (NG // 2) + ql
                        out_sb = opool.tile(
                            [128, GW], f32, name=f"osb{qb}", tag="osb"
                        )
                        if ph == 1 and ql % 2 == 1:
                            nc.vector.tensor_scalar_add(
                                out_sb, psums[qb], bias_sb[:, 0:1])
                        else:
                            nc.scalar.activation(
                                out_sb,
                                psums[qb],
                                mybir.ActivationFunctionType.Identity,
                                bias=bias_sb[:, 0:1],
                                scale=1.0,
                            )
                        dma_eng = nc.scalar if ql % 2 == 0 else nc.sync
                        dma_eng.dma_start(
                            out=out_ap[:, qb * bpg:(qb + 1) * bpg, :],
                            in_=out_sb)

    nc.compile()
    return nc


def _prep_host(x0, xk, W, b):
    """Host-side layout prep (no arithmetic): shard, transpose, replicate."""
    part = np.arange(128)
    hh = (part % HB)[None, :] + HB * np.arange(NHB)[:, None]   # [NHB, 128]
    mm = (part // HB)[None, :] + MG * np.arange(NMG)[:, None]  # [NMG, 128]

    Wr = W.reshape(C, H, M)
    # w3[:, k*C:(k+1)*C] = lhsT for chunk CK[k]; [128, NCHUNK*C] layout
    # so each load quarter is contiguous per partition.
    w3 = np.empty((128, NCHUNK * C), ml_dtypes.bfloat16)
    for k, (g, j) in enumerate(CK):
        w3[:, k * C:(k + 1) * C] = Wr[:, hh[j], mm[g]].T.astype(
            ml_dtypes.bfloat16)
    bias = np.ascontiguousarray(b.reshape(C, 1)).astype(np.float32)

    in_maps = []
    for kcore in range(N_CORES):
        x0s = x0[kcore * BC:(kcore + 1) * BC]    # [BC, M, D]
        xks = xk[kcore * BC:(kcore + 1) * BC]    # [BC, H, D]
        xk2 = (
            np.ascontiguousarray(xks.transpose(1, 0, 2))
            .reshape(H, COLS)
            .astype(ml_dtypes.bfloat16)
        )
        x02 = (
            np.ascontiguousarray(x0s.transpose(1, 0, 2))
            .reshape(M, COLS)
            .astype(ml_dtypes.bfloat16)
        )
        in_maps.append(
            {
                "xkrep_in": np.ascontiguousarray(xk2[hh]),
                "x0bc_in": np.ascontiguousarray(x02[mm]),
                "w3_in": w3,
                "bias_in": bias,
            }
        )
    return in_maps


def _run(in_maps, **kwargs):
    from concourse import bass_utils

    if "nc" not in _cache:
        _cache["nc"] = _build()
    return bass_utils.run_bass_kernel_spmd(
        _cache["nc"], in_maps, core_ids=list(range(N_CORES)), **kwargs
    )


def kernel(x0, xk, W, b, _bench=[None]):
    x0 = np.asarray(x0, dtype=np.float32)
    xk = np.asarray(xk, dtype=np.float32)
    W = np.asarray(W, dtype=np.float32)
    b = np.asarray(b, dtype=np.float32)
    in_maps = _prep_host(x0, xk, W, b)
    res = _run(in_maps)
    _bench[0] = res
    # per-core out is [C, BC, D]; concatenate batches then put C second
    out = np.concatenate([r["out"] for r in res.results], axis=1)
    return np.ascontiguousarray(out.transpose(1, 0, 2)).astype(
        np.float32, copy=False)
